# revision 47
# baseline (speedup 1.0000x reference)
"""Additive (Bahdanau) attention scoring kernel for Trainium2, 8-core SPMD.

Reference computation (B=16, S=4096, D=1024, all fp32):
    q      = target @ Wq.T                    # [B, D]
    k      = memory @ Wk.T                    # [B, S, D]
    scores = tanh(q[:, None, :] + k) @ v      # [B, S]
    out    = softmax(scores - 1e9 * mask, axis=-1)

Sharding: batch across the 8 cores (2 batches per core), weights replicated.

Host-side prep (layout + bf16 rounding): masked positions contribute exactly
0 to the reference softmax (exp(-1e9) == 0 in fp32), so memory is compacted
to the unmasked columns per batch (padded to a 128 multiple with duplicates
of the first kept column; pads are zeroed on device via padmask before the
softmax sum). Layout is strip-blocked [P, dc-major] so each strip is ONE
contiguous-per-partition DMA. Matmul operands are shipped as bf16 (validated
3.3e-3 max rel err vs the 2e-2 gate); all accumulation stays f32 on device.
The compact softmax result is unscattered to full S on the host (inverse of
the input gather).

Device design (v3 — s-on-partitions, fused q):
  - k^T tiles [s=128, e=512] = memchunk.T @ Wk chunk, accumulated over the
    8 d-chunks in PSUM (stationary = mem chunk, moving = Wk, both bf16).
    s lands on PSUM partitions.
  - q is folded into the same PSUM accumulation as a final K=2 matmul:
    sel[:, b] selects batch b's row of q_row [2, 1024] and broadcasts it
    across all 128 partitions. No DVE add, no PSUM->SBUF staging.
  - ACT: tt = tanh(psum) directly; DVE: fused (tt * v) + row-sum via
    scalar_tensor_tensor, one op per e-half, partials combined per block.
  - Softmax finale per batch on compact [128, NSQ] layout: exp (ACT),
    padmask multiply + row-sum (DVE), ones-matmul partition reduce (PE),
    reciprocal + scale (DVE). No max-shift needed: |scores| <= sum|v| ~ 8.
  - DMA: sync queue = Wq first (q matmuls lead the PE stream), then Wk and
    consts; scalar queue = all memory strips. No on-device casts.
"""

import os
from contextlib import ExitStack

import numpy as np
import ml_dtypes

import concourse.tile as tile
from concourse import bacc, mybir
import concourse.bass as bass

B, S, D = 16, 4096, 1024
N_CORES = 8
NB = B // N_CORES  # batches per core
P = 128
DC = D // P        # contraction chunks (8)
EH = 2             # e halves (2 x 512)
SW = 512           # max strip width along compacted s

F32 = mybir.dt.float32
BF16 = mybir.dt.bfloat16
AF = mybir.ActivationFunctionType
ALU = mybir.AluOpType
NPBF16 = ml_dtypes.bfloat16

_CACHE = {}
_UNSCATTER = {}


def make_widths(max_kept):
    """Strip widths covering max_kept compacted columns (128-granular).
    Two small leading strips let the PE start before the big DMAs land."""
    total = max(256, ((max_kept + 127) // 128) * 128)
    ws = []
    rem = total
    for wt in (128, 256):
        if rem >= wt + 128:
            ws.append(wt)
            rem -= wt
    while rem > SW:
        ws.append(SW)
        rem -= SW
    if rem:
        ws.append(rem)
    return tuple(ws)


def _build_program(stage, widths):
    s_pad = sum(widths)
    NSQ = s_pad // P

    nc = bacc.Bacc("TRN2", target_bir_lowering=False, debug=False)

    memS = nc.dram_tensor("memS", [NB, P, DC * s_pad], BF16, kind="ExternalInput").ap()
    wk2 = nc.dram_tensor("wk2", [P, DC * D], BF16, kind="ExternalInput").ap()
    wq2 = nc.dram_tensor("wq2", [P, DC * D], BF16, kind="ExternalInput").ap()
    tgt2 = nc.dram_tensor("tgt2", [P, DC * NB], BF16, kind="ExternalInput").ap()
    vrep = nc.dram_tensor("vrep", [P, D], BF16, kind="ExternalInput").ap()
    padm = nc.dram_tensor("padm", [NB, P, NSQ], F32, kind="ExternalInput").ap()
    selm = nc.dram_tensor("selm", [NB, NB * P], BF16, kind="ExternalInput").ap()
    out = nc.dram_tensor("out", [NB, P, NSQ], F32, kind="ExternalOutput").ap()

    with tile.TileContext(nc) as tc, ExitStack() as ctx:
        consts = ctx.enter_context(tc.tile_pool(name="consts", bufs=1))
        memb_pool = ctx.enter_context(tc.tile_pool(name="memb", bufs=4))
        tt_pool = ctx.enter_context(tc.tile_pool(name="tt", bufs=6))
        scr_pool = ctx.enter_context(tc.tile_pool(name="scr", bufs=2))
        s0_pool = ctx.enter_context(tc.tile_pool(name="s0", bufs=4))
        score_pool = ctx.enter_context(tc.tile_pool(name="score", bufs=2))
        fin_pool = ctx.enter_context(tc.tile_pool(name="fin", bufs=2))
        kps_pool = ctx.enter_context(tc.tile_pool(name="kps", bufs=6, space="PSUM"))
        sm_pool = ctx.enter_context(tc.tile_pool(name="smps", bufs=2, space="PSUM"))

        strips = []
        off = 0
        for w in widths:
            strips.append((off, w))
            off += w

        # ---- sync queue: tgt + sel + Wk (the k-matmuls lead the PE
        # program), then the small consts. Wq goes on the scalar queue
        # between the early strips (q matmuls are emitted after the first
        # QDEF s-blocks, whose q-folds are deferred).
        # PE warm-up: ~3.4us of matmuls on a zeroed tile while the first
        # DMAs land, so the HAM activity window reaches K=8/8 (2.4 GHz)
        # before the real stream starts.
        wrm = consts.tile([P, SW], BF16)
        nc.vector.memset(wrm[:], 0.0)
        warm_ps = sm_pool.tile([P, SW], F32, tag="small", name="warm_ps")
        for _ in range(8):
            nc.tensor.matmul(warm_ps[:], wrm[:, :P], wrm[:], start=True, stop=True)
        # Wk chunks alternate queues in consumption order (dc0 first) so
        # the k-matmul stream is never paced by a single queue; strip0
        # leads the scalar queue. Wq follows as 2 big DMAs on sync -- the
        # q matmuls are emitted after QDEF s-blocks and never stall.
        wk_b = consts.tile([P, DC * D], BF16)
        wq_b = consts.tile([P, DC * D], BF16)
        mem_first = memb_pool.tile([P, DC * SW], BF16, tag="memb", name="mem_t")
        w0 = widths[0]
        nc.scalar.dma_start(mem_first[:, :DC * w0], memS[0, :, :DC * w0])
        for dc in range(DC):
            eng = nc.sync if dc % 2 == 0 else nc.scalar
            eng.dma_start(wk_b[:, dc * D:(dc + 1) * D], wk2[:, dc * D:(dc + 1) * D])
        tgt_b = consts.tile([P, DC * NB], BF16)
        nc.sync.dma_start(tgt_b[:], tgt2[:, :])
        sel = consts.tile([NB, NB * P], BF16)
        nc.sync.dma_start(sel[:], selm[:, :])
        for h in range(2):
            nc.sync.dma_start(
                wq_b[:, h * 4 * D:(h + 1) * 4 * D],
                wq2[:, h * 4 * D:(h + 1) * 4 * D],
            )
        v_b = consts.tile([P, D], BF16)
        nc.sync.dma_start(v_b[:], vrep[:, :])
        pad_sb = consts.tile([P, NB * NSQ], F32)
        for b in range(NB):
            nc.sync.dma_start(pad_sb[:, b * NSQ:(b + 1) * NSQ], padm[b])
        ones_sb = consts.tile([P, P], F32)
        nc.vector.memset(ones_sb[:], 1.0)

        q_row = consts.tile([NB, D], BF16)

        def emit_q():
            # q[b, e] = sum_d target[b, d] * Wq[e, d], row layout [2, 1024]
            for j in range(EH):
                q_ps = sm_pool.tile([NB, SW], F32, tag="small", name="q_ps")
                for dc in range(DC):
                    nc.tensor.matmul(
                        q_ps[:],
                        tgt_b[:, dc * NB:(dc + 1) * NB],
                        wq_b[:, dc * D + j * SW: dc * D + (j + 1) * SW],
                        start=(dc == 0),
                        stop=(dc == DC - 1),
                    )
                nc.vector.tensor_copy(q_row[:, j * SW:(j + 1) * SW], q_ps[:])

        def emit_score(b, jg, kps, score_sb):
            s0t = s0_pool.tile([P, EH], F32, tag="s0", name="s0t")
            for eh in range(EH):
                tt = tt_pool.tile([P, SW], BF16, tag="tt", name="tt")
                nc.scalar.activation(tt[:], kps[eh][:], AF.Tanh)
                if stage < 3:
                    if eh == 0 and jg == NSQ - 1:
                        dbg = fin_pool.tile([P, NSQ], F32, tag="outt", name="dbg")
                        nc.vector.tensor_copy(dbg[:], tt[:, :NSQ])
                        nc.sync.dma_start(out[b], dbg[:])
                    continue
                # fused (tt * v) + row-sum in one native DVE op
                scr = scr_pool.tile([P, SW], BF16, tag="scr", name="scr")
                nc.vector.scalar_tensor_tensor(
                    out=scr[:],
                    in0=tt[:],
                    scalar=0.0,
                    in1=v_b[:, eh * SW:(eh + 1) * SW],
                    op0=ALU.add,
                    op1=ALU.mult,
                    accum_out=s0t[:, eh:eh + 1],
                )
            if stage >= 3:
                nc.vector.tensor_add(
                    score_sb[:, jg:jg + 1], s0t[:, 0:1], s0t[:, 1:2]
                )

        def emit_qfold(b, jg, kps, score_sb):
            # fold q into the accumulation: psum[s, e] += q[b, e]
            for eh in range(EH):
                nc.tensor.matmul(
                    kps[eh][:],
                    sel[:, b * P:(b + 1) * P],
                    q_row[:, eh * SW:(eh + 1) * SW],
                    start=False,
                    stop=True,
                )
            emit_score(b, jg, kps, score_sb)

        QDEF = 3  # s-blocks whose q-fold is deferred until Wq has landed
        sblk = 0
        q_emitted = False
        pending = []
        for b in range(NB):
            score_sb = score_pool.tile([P, NSQ], F32, tag="score", name="score_sb")
            # process the tail strip early so the batch ends on a full strip
            order = list(range(len(strips)))
            if len(order) > 2:
                order = [order[0], order[-1]] + order[1:-1]
            for si in order:
                off, w = strips[si]
                if b == 0 and si == order[0]:
                    mem_t = mem_first
                else:
                    mem_t = memb_pool.tile([P, DC * SW], BF16, tag="memb", name="mem_t")
                    nc.scalar.dma_start(mem_t[:, :DC * w], memS[b, :, DC * off:DC * (off + w)])
                for jj in range(w // P):
                    if sblk == QDEF and not q_emitted:
                        emit_q()
                        for args in pending:
                            emit_qfold(*args)
                        pending.clear()
                        q_emitted = True
                    jg = off // P + jj
                    kps = [
                        kps_pool.tile([P, SW], F32, tag="k", name="k_ps")
                        for _ in range(EH)
                    ]
                    for dc in range(DC):
                        stat = mem_t[:, dc * w + jj * P: dc * w + (jj + 1) * P]
                        for eh in range(EH):
                            nc.tensor.matmul(
                                kps[eh][:],
                                stat,
                                wk_b[:, dc * D + eh * SW: dc * D + (eh + 1) * SW],
                                start=(dc == 0),
                                stop=False,
                            )
                    if q_emitted:
                        emit_qfold(b, jg, kps, score_sb)
                    else:
                        pending.append((b, jg, kps, score_sb))
                    sblk += 1
            if stage < 3:
                continue
            if stage < 25:
                outt = fin_pool.tile([P, NSQ], F32, tag="outt", name="outt")
                nc.vector.tensor_copy(outt[:], score_sb[:])
                nc.sync.dma_start(out[b], outt[:])
                continue
            # ---- masked softmax finale for batch b (compact layout) ----
            esq = fin_pool.tile([P, NSQ], F32, tag="esq", name="esq")
            nc.scalar.activation(esq[:], score_sb[:], AF.Exp)
            em = fin_pool.tile([P, NSQ], F32, tag="em", name="em")
            part = fin_pool.tile([P, 1], F32, tag="part", name="part")
            nc.vector.tensor_mul(em[:], esq[:], pad_sb[:, b * NSQ:(b + 1) * NSQ])
            nc.vector.reduce_sum(part[:], em[:], axis=mybir.AxisListType.X)
            tot_ps = sm_pool.tile([P, 1], F32, tag="small", name="tot_ps")
            nc.tensor.matmul(tot_ps[:], ones_sb[:], part[:], start=True, stop=True)
            recip = fin_pool.tile([P, 1], F32, tag="recip", name="recip")
            nc.vector.reciprocal(recip[:], tot_ps[:])
            outt = fin_pool.tile([P, NSQ], F32, tag="outt", name="outt")
            nc.vector.tensor_scalar_mul(outt[:], em[:], recip[:, 0:1])
            nc.sync.dma_start(out[b], outt[:])

    nc.compile()
    return nc


def get_program(stage=None, widths=None):
    if stage is None:
        stage = int(os.environ.get("KERNEL_STAGE", "27"))
    assert widths is not None
    key = (stage, widths)
    if key not in _CACHE:
        _CACHE[key] = _build_program(stage, widths)
    return _CACHE[key]


def prepare_in_maps(memory, target, memory_mask, Wq, Wk, v):
    memory = np.asarray(memory, dtype=np.float32)
    target = np.asarray(target, dtype=np.float32)
    Wq = np.asarray(Wq, dtype=np.float32)
    Wk = np.asarray(Wk, dtype=np.float32)
    v = np.asarray(v, dtype=np.float32)
    mask = np.asarray(memory_mask)

    # host-side sharding / layout prep
    keep_bool = ~mask                                                # [B, S]
    n_kept = keep_bool.sum(1).astype(np.int64)
    widths = make_widths(int(n_kept.max()))
    s_pad = sum(widths)
    NSQ = s_pad // P

    memT = memory.transpose(0, 2, 1)                                 # [B, D, S] view
    kept_idx = []
    kept_pad = np.empty((B, s_pad), dtype=np.int64)
    for b in range(B):
        k = np.flatnonzero(keep_bool[b])
        kept_idx.append(k)
        kept_pad[b, :len(k)] = k
        kept_pad[b, len(k):] = k[0]  # pad data: duplicate first kept column

    # memS[b, p, dc-major strip layout]: strip at offset `off`, width w
    # occupies columns [DC*off, DC*(off+w)), internally dc*w + s.
    memS = np.empty((B, P, DC * s_pad), dtype=NPBF16)
    for b in range(B):
        memC = memT[b][:, kept_pad[b]].astype(NPBF16)                # [D, s_pad]
        memC4 = memC.reshape(DC, P, s_pad)
        off = 0
        for w in widths:
            blk = memC4[:, :, off:off + w]                           # [DC, P, w]
            memS[b, :, DC * off:DC * (off + w)] = (
                blk.transpose(1, 0, 2).reshape(P, DC * w))
            off += w

    # padmask in compact [P, NSQ] layout: compact index c = j*128 + p
    padm = np.zeros((B, P, NSQ), dtype=np.float32)
    for b in range(B):
        c = np.arange(s_pad)
        keepc = (c < n_kept[b]).astype(np.float32)
        padm[b] = keepc.reshape(NSQ, P).T

    # w2[p, dc*D + e] = W[e, dc*128 + p] -- SBUF-layout weight images
    wk2 = np.ascontiguousarray(
        Wk.T.reshape(DC, P, D).transpose(1, 0, 2).reshape(P, DC * D)).astype(NPBF16)
    wq2 = np.ascontiguousarray(
        Wq.T.reshape(DC, P, D).transpose(1, 0, 2).reshape(P, DC * D)).astype(NPBF16)
    # tgt2[p, dc*NB+b] = target[b, dc*128+p], per core slice of batches
    tgtT = target.T.reshape(DC, P, B).astype(NPBF16)                 # [DC, P, B]
    vr = np.ascontiguousarray(np.broadcast_to(v[None, :], (P, D))).astype(NPBF16)

    selm = np.zeros((NB, NB * P), dtype=NPBF16)
    for b in range(NB):
        selm[b, b * P:(b + 1) * P] = 1.0

    _UNSCATTER.clear()
    _UNSCATTER["kept_idx"] = kept_idx
    _UNSCATTER["n_kept"] = n_kept
    _UNSCATTER["s_pad"] = s_pad

    in_maps = [
        {
            "memS": np.ascontiguousarray(memS[c * NB:(c + 1) * NB]),
            "wk2": wk2,
            "wq2": wq2,
            "tgt2": np.ascontiguousarray(
                tgtT[:, :, c * NB:(c + 1) * NB].transpose(1, 0, 2).reshape(P, DC * NB)),
            "vrep": vr,
            "padm": np.ascontiguousarray(padm[c * NB:(c + 1) * NB]),
            "selm": selm,
        }
        for c in range(N_CORES)
    ]
    return in_maps, widths


def unscatter_batch(out_b, batch):
    """out_b: [P, NSQ] compact normalized softmax for global batch index."""
    flat = np.asarray(out_b).T.ravel()
    full = np.zeros(S, dtype=np.float32)
    k = _UNSCATTER["kept_idx"][batch]
    full[k] = flat[:len(k)]
    return full


def gather_output(results):
    out = np.empty((B, S), dtype=np.float32)
    for c in range(N_CORES):
        o = results[c]["out"]
        for nb in range(NB):
            out[c * NB + nb] = unscatter_batch(o[nb], c * NB + nb)
    return out


def kernel(memory, target, memory_mask, Wq, Wk, v):
    from concourse.bass_utils import run_bass_kernel_spmd

    in_maps, widths = prepare_in_maps(memory, target, memory_mask, Wq, Wk, v)
    nc = get_program(widths=widths)
    res = run_bass_kernel_spmd(nc, in_maps, list(range(N_CORES)))
    return gather_output(res.results)


# revision 49
# speedup vs baseline: 1.0514x; 1.0514x over previous
"""Additive (Bahdanau) attention scoring kernel for Trainium2, 8-core SPMD.

Reference computation (B=16, S=4096, D=1024, all fp32):
    q      = target @ Wq.T                    # [B, D]
    k      = memory @ Wk.T                    # [B, S, D]
    scores = tanh(q[:, None, :] + k) @ v      # [B, S]
    out    = softmax(scores - 1e9 * mask, axis=-1)

Sharding: batch across the 8 cores (2 batches per core), weights replicated.

Host-side prep (layout + bf16 rounding): masked positions contribute exactly
0 to the reference softmax (exp(-1e9) == 0 in fp32), so memory is compacted
to the unmasked columns per batch (padded to a 128 multiple with duplicates
of the first kept column; pads are zeroed on device via padmask before the
softmax sum). Layout is strip-blocked [P, dc-major] so each strip is ONE
contiguous-per-partition DMA. Matmul operands are shipped as bf16 (validated
3.3e-3 max rel err vs the 2e-2 gate); all accumulation stays f32 on device.
The compact softmax result is unscattered to full S on the host (inverse of
the input gather).

Device design (v3 — s-on-partitions, fused q):
  - k^T tiles [s=128, e=512] = memchunk.T @ Wk chunk, accumulated over the
    8 d-chunks in PSUM (stationary = mem chunk, moving = Wk, both bf16).
    s lands on PSUM partitions.
  - q is folded into the same PSUM accumulation as a final K=2 matmul:
    sel[:, b] selects batch b's row of q_row [2, 1024] and broadcasts it
    across all 128 partitions. No DVE add, no PSUM->SBUF staging.
  - ACT: tt = tanh(psum) directly; DVE: fused (tt * v) + row-sum via
    scalar_tensor_tensor, one op per e-half, partials combined per block.
  - Softmax finale per batch on compact [128, NSQ] layout: exp (ACT),
    padmask multiply + row-sum (DVE), ones-matmul partition reduce (PE),
    reciprocal + scale (DVE). No max-shift needed: |scores| <= sum|v| ~ 8.
  - DMA: sync queue = Wq first (q matmuls lead the PE stream), then Wk and
    consts; scalar queue = all memory strips. No on-device casts.
"""

import os
from contextlib import ExitStack

import numpy as np
import ml_dtypes

import concourse.tile as tile
from concourse import bacc, mybir
import concourse.bass as bass

B, S, D = 16, 4096, 1024
N_CORES = 8
NB = B // N_CORES  # batches per core
P = 128
DC = D // P        # contraction chunks (8)
EH = 2             # e halves (2 x 512)
SW = 512           # max strip width along compacted s

F32 = mybir.dt.float32
BF16 = mybir.dt.bfloat16
AF = mybir.ActivationFunctionType
ALU = mybir.AluOpType
NPBF16 = ml_dtypes.bfloat16

_CACHE = {}
_UNSCATTER = {}


def make_widths(max_kept):
    """Strip widths covering max_kept compacted columns (128-granular).
    Two small leading strips let the PE start before the big DMAs land."""
    total = max(256, ((max_kept + 127) // 128) * 128)
    ws = []
    rem = total
    for wt in (256, 256):
        if rem >= wt + 128:
            ws.append(wt)
            rem -= wt
    while rem > SW:
        ws.append(SW)
        rem -= SW
    if rem:
        ws.append(rem)
    return tuple(ws)


def _build_program(stage, widths):
    s_pad = sum(widths)
    NSQ = s_pad // P

    nc = bacc.Bacc("TRN2", target_bir_lowering=False, debug=False)

    memS = nc.dram_tensor("memS", [NB, P, DC * s_pad], BF16, kind="ExternalInput").ap()
    wk2 = nc.dram_tensor("wk2", [P, DC * D], BF16, kind="ExternalInput").ap()
    wq2 = nc.dram_tensor("wq2", [P, DC * D], BF16, kind="ExternalInput").ap()
    tgt2 = nc.dram_tensor("tgt2", [P, DC * NB], BF16, kind="ExternalInput").ap()
    vrep = nc.dram_tensor("vrep", [P, D], BF16, kind="ExternalInput").ap()
    padm = nc.dram_tensor("padm", [NB, P, NSQ], F32, kind="ExternalInput").ap()
    selm = nc.dram_tensor("selm", [NB, NB * P], BF16, kind="ExternalInput").ap()
    out = nc.dram_tensor("out", [NB, P, NSQ], F32, kind="ExternalOutput").ap()

    with tile.TileContext(nc) as tc, ExitStack() as ctx:
        consts = ctx.enter_context(tc.tile_pool(name="consts", bufs=1))
        memb_pool = ctx.enter_context(tc.tile_pool(name="memb", bufs=4))
        tt_pool = ctx.enter_context(tc.tile_pool(name="tt", bufs=6))
        scr_pool = ctx.enter_context(tc.tile_pool(name="scr", bufs=2))
        s0_pool = ctx.enter_context(tc.tile_pool(name="s0", bufs=4))
        score_pool = ctx.enter_context(tc.tile_pool(name="score", bufs=2))
        fin_pool = ctx.enter_context(tc.tile_pool(name="fin", bufs=2))
        kps_pool = ctx.enter_context(tc.tile_pool(name="kps", bufs=6, space="PSUM"))
        sm_pool = ctx.enter_context(tc.tile_pool(name="smps", bufs=2, space="PSUM"))

        strips = []
        off = 0
        for w in widths:
            strips.append((off, w))
            off += w

        # ---- sync queue: tgt + sel + Wk (the k-matmuls lead the PE
        # program), then the small consts. Wq goes on the scalar queue
        # between the early strips (q matmuls are emitted after the first
        # QDEF s-blocks, whose q-folds are deferred).
        # Wk chunks alternate queues in consumption order (dc0 first) so
        # the k-matmul stream is never paced by a single queue; strip0
        # leads the scalar queue. Wq follows as 2 big DMAs on sync -- the
        # q matmuls are emitted after QDEF s-blocks and never stall.
        wk_b = consts.tile([P, DC * D], BF16)
        wq_b = consts.tile([P, DC * D], BF16)
        mem_first = memb_pool.tile([P, DC * SW], BF16, tag="memb", name="mem_t")
        w0 = widths[0]
        nc.scalar.dma_start(mem_first[:, :DC * w0], memS[0, :, :DC * w0])
        for dc in range(DC):
            eng = nc.sync if dc % 2 == 0 else nc.scalar
            eng.dma_start(wk_b[:, dc * D:(dc + 1) * D], wk2[:, dc * D:(dc + 1) * D])
        tgt_b = consts.tile([P, DC * NB], BF16)
        nc.sync.dma_start(tgt_b[:], tgt2[:, :])
        sel = consts.tile([NB, NB * P], BF16)
        nc.sync.dma_start(sel[:], selm[:, :])
        for h in range(2):
            nc.sync.dma_start(
                wq_b[:, h * 4 * D:(h + 1) * 4 * D],
                wq2[:, h * 4 * D:(h + 1) * 4 * D],
            )
        v_b = consts.tile([P, D], BF16)
        nc.sync.dma_start(v_b[:], vrep[:, :])
        pad_sb = consts.tile([P, NB * NSQ], F32)
        for b in range(NB):
            nc.sync.dma_start(pad_sb[:, b * NSQ:(b + 1) * NSQ], padm[b])
        ones_sb = consts.tile([P, P], F32)
        nc.vector.memset(ones_sb[:], 1.0)

        q_row = consts.tile([NB, D], BF16)

        def emit_q():
            # q[b, e] = sum_d target[b, d] * Wq[e, d], row layout [2, 1024]
            for j in range(EH):
                q_ps = sm_pool.tile([NB, SW], F32, tag="small", name="q_ps")
                for dc in range(DC):
                    nc.tensor.matmul(
                        q_ps[:],
                        tgt_b[:, dc * NB:(dc + 1) * NB],
                        wq_b[:, dc * D + j * SW: dc * D + (j + 1) * SW],
                        start=(dc == 0),
                        stop=(dc == DC - 1),
                    )
                nc.vector.tensor_copy(q_row[:, j * SW:(j + 1) * SW], q_ps[:])

        def emit_score(b, jg, kps, score_sb):
            s0t = s0_pool.tile([P, EH], F32, tag="s0", name="s0t")
            for eh in range(EH):
                tt = tt_pool.tile([P, SW], BF16, tag="tt", name="tt")
                nc.scalar.activation(tt[:], kps[eh][:], AF.Tanh)
                if stage < 3:
                    if eh == 0 and jg == NSQ - 1:
                        dbg = fin_pool.tile([P, NSQ], F32, tag="outt", name="dbg")
                        nc.vector.tensor_copy(dbg[:], tt[:, :NSQ])
                        nc.sync.dma_start(out[b], dbg[:])
                    continue
                # fused (tt * v) + row-sum in one native DVE op
                scr = scr_pool.tile([P, SW], BF16, tag="scr", name="scr")
                nc.vector.scalar_tensor_tensor(
                    out=scr[:],
                    in0=tt[:],
                    scalar=0.0,
                    in1=v_b[:, eh * SW:(eh + 1) * SW],
                    op0=ALU.add,
                    op1=ALU.mult,
                    accum_out=s0t[:, eh:eh + 1],
                )
            if stage >= 3:
                nc.vector.tensor_add(
                    score_sb[:, jg:jg + 1], s0t[:, 0:1], s0t[:, 1:2]
                )

        def emit_qfold(b, jg, kps, score_sb):
            # fold q into the accumulation: psum[s, e] += q[b, e]
            for eh in range(EH):
                nc.tensor.matmul(
                    kps[eh][:],
                    sel[:, b * P:(b + 1) * P],
                    q_row[:, eh * SW:(eh + 1) * SW],
                    start=False,
                    stop=True,
                )
            emit_score(b, jg, kps, score_sb)

        QDEF = 3  # s-blocks whose q-fold is deferred until Wq has landed
        sblk = 0
        q_emitted = False
        pending = []
        for b in range(NB):
            score_sb = score_pool.tile([P, NSQ], F32, tag="score", name="score_sb")
            # process the tail strip early so the batch ends on a full strip
            order = list(range(len(strips)))
            if len(order) > 2:
                order = [order[0], order[-1]] + order[1:-1]
            for si in order:
                off, w = strips[si]
                if b == 0 and si == order[0]:
                    mem_t = mem_first
                else:
                    mem_t = memb_pool.tile([P, DC * SW], BF16, tag="memb", name="mem_t")
                    nc.scalar.dma_start(mem_t[:, :DC * w], memS[b, :, DC * off:DC * (off + w)])
                for jj in range(w // P):
                    if sblk == QDEF and not q_emitted:
                        emit_q()
                        for args in pending:
                            emit_qfold(*args)
                        pending.clear()
                        q_emitted = True
                    jg = off // P + jj
                    kps = [
                        kps_pool.tile([P, SW], F32, tag="k", name="k_ps")
                        for _ in range(EH)
                    ]
                    for dc in range(DC):
                        stat = mem_t[:, dc * w + jj * P: dc * w + (jj + 1) * P]
                        for eh in range(EH):
                            nc.tensor.matmul(
                                kps[eh][:],
                                stat,
                                wk_b[:, dc * D + eh * SW: dc * D + (eh + 1) * SW],
                                start=(dc == 0),
                                stop=False,
                            )
                    if q_emitted:
                        emit_qfold(b, jg, kps, score_sb)
                    else:
                        pending.append((b, jg, kps, score_sb))
                    sblk += 1
            if stage < 3:
                continue
            if stage < 25:
                outt = fin_pool.tile([P, NSQ], F32, tag="outt", name="outt")
                nc.vector.tensor_copy(outt[:], score_sb[:])
                nc.sync.dma_start(out[b], outt[:])
                continue
            # ---- masked softmax finale for batch b (compact layout) ----
            esq = fin_pool.tile([P, NSQ], F32, tag="esq", name="esq")
            nc.scalar.activation(esq[:], score_sb[:], AF.Exp)
            em = fin_pool.tile([P, NSQ], F32, tag="em", name="em")
            part = fin_pool.tile([P, 1], F32, tag="part", name="part")
            nc.vector.tensor_mul(em[:], esq[:], pad_sb[:, b * NSQ:(b + 1) * NSQ])
            nc.vector.reduce_sum(part[:], em[:], axis=mybir.AxisListType.X)
            tot_ps = sm_pool.tile([P, 1], F32, tag="small", name="tot_ps")
            nc.tensor.matmul(tot_ps[:], ones_sb[:], part[:], start=True, stop=True)
            recip = fin_pool.tile([P, 1], F32, tag="recip", name="recip")
            nc.vector.reciprocal(recip[:], tot_ps[:])
            outt = fin_pool.tile([P, NSQ], F32, tag="outt", name="outt")
            nc.vector.tensor_scalar_mul(outt[:], em[:], recip[:, 0:1])
            nc.sync.dma_start(out[b], outt[:])

    nc.compile()
    return nc


def get_program(stage=None, widths=None):
    if stage is None:
        stage = int(os.environ.get("KERNEL_STAGE", "27"))
    assert widths is not None
    key = (stage, widths)
    if key not in _CACHE:
        _CACHE[key] = _build_program(stage, widths)
    return _CACHE[key]


def prepare_in_maps(memory, target, memory_mask, Wq, Wk, v):
    memory = np.asarray(memory, dtype=np.float32)
    target = np.asarray(target, dtype=np.float32)
    Wq = np.asarray(Wq, dtype=np.float32)
    Wk = np.asarray(Wk, dtype=np.float32)
    v = np.asarray(v, dtype=np.float32)
    mask = np.asarray(memory_mask)

    # host-side sharding / layout prep
    keep_bool = ~mask                                                # [B, S]
    n_kept = keep_bool.sum(1).astype(np.int64)
    widths = make_widths(int(n_kept.max()))
    s_pad = sum(widths)
    NSQ = s_pad // P

    memT = memory.transpose(0, 2, 1)                                 # [B, D, S] view
    kept_idx = []
    kept_pad = np.empty((B, s_pad), dtype=np.int64)
    for b in range(B):
        k = np.flatnonzero(keep_bool[b])
        kept_idx.append(k)
        kept_pad[b, :len(k)] = k
        kept_pad[b, len(k):] = k[0]  # pad data: duplicate first kept column

    # memS[b, p, dc-major strip layout]: strip at offset `off`, width w
    # occupies columns [DC*off, DC*(off+w)), internally dc*w + s.
    memS = np.empty((B, P, DC * s_pad), dtype=NPBF16)
    for b in range(B):
        memC = memT[b][:, kept_pad[b]].astype(NPBF16)                # [D, s_pad]
        memC4 = memC.reshape(DC, P, s_pad)
        off = 0
        for w in widths:
            blk = memC4[:, :, off:off + w]                           # [DC, P, w]
            memS[b, :, DC * off:DC * (off + w)] = (
                blk.transpose(1, 0, 2).reshape(P, DC * w))
            off += w

    # padmask in compact [P, NSQ] layout: compact index c = j*128 + p
    padm = np.zeros((B, P, NSQ), dtype=np.float32)
    for b in range(B):
        c = np.arange(s_pad)
        keepc = (c < n_kept[b]).astype(np.float32)
        padm[b] = keepc.reshape(NSQ, P).T

    # w2[p, dc*D + e] = W[e, dc*128 + p] -- SBUF-layout weight images
    wk2 = np.ascontiguousarray(
        Wk.T.reshape(DC, P, D).transpose(1, 0, 2).reshape(P, DC * D)).astype(NPBF16)
    wq2 = np.ascontiguousarray(
        Wq.T.reshape(DC, P, D).transpose(1, 0, 2).reshape(P, DC * D)).astype(NPBF16)
    # tgt2[p, dc*NB+b] = target[b, dc*128+p], per core slice of batches
    tgtT = target.T.reshape(DC, P, B).astype(NPBF16)                 # [DC, P, B]
    vr = np.ascontiguousarray(np.broadcast_to(v[None, :], (P, D))).astype(NPBF16)

    selm = np.zeros((NB, NB * P), dtype=NPBF16)
    for b in range(NB):
        selm[b, b * P:(b + 1) * P] = 1.0

    _UNSCATTER.clear()
    _UNSCATTER["kept_idx"] = kept_idx
    _UNSCATTER["n_kept"] = n_kept
    _UNSCATTER["s_pad"] = s_pad

    in_maps = [
        {
            "memS": np.ascontiguousarray(memS[c * NB:(c + 1) * NB]),
            "wk2": wk2,
            "wq2": wq2,
            "tgt2": np.ascontiguousarray(
                tgtT[:, :, c * NB:(c + 1) * NB].transpose(1, 0, 2).reshape(P, DC * NB)),
            "vrep": vr,
            "padm": np.ascontiguousarray(padm[c * NB:(c + 1) * NB]),
            "selm": selm,
        }
        for c in range(N_CORES)
    ]
    return in_maps, widths


def unscatter_batch(out_b, batch):
    """out_b: [P, NSQ] compact normalized softmax for global batch index."""
    flat = np.asarray(out_b).T.ravel()
    full = np.zeros(S, dtype=np.float32)
    k = _UNSCATTER["kept_idx"][batch]
    full[k] = flat[:len(k)]
    return full


def gather_output(results):
    out = np.empty((B, S), dtype=np.float32)
    for c in range(N_CORES):
        o = results[c]["out"]
        for nb in range(NB):
            out[c * NB + nb] = unscatter_batch(o[nb], c * NB + nb)
    return out


def kernel(memory, target, memory_mask, Wq, Wk, v):
    from concourse.bass_utils import run_bass_kernel_spmd

    in_maps, widths = prepare_in_maps(memory, target, memory_mask, Wq, Wk, v)
    nc = get_program(widths=widths)
    res = run_bass_kernel_spmd(nc, in_maps, list(range(N_CORES)))
    return gather_output(res.results)
